# revision 1
# baseline (speedup 1.0000x reference)
"""Trainium2 Bass kernel for CGCalculatorSingle (segment_reduce).

Computes out[b,f,mu[k]] += C[k] * X1[b,f,m1[k]] * X2[b,f,m2[k]] for k in [0,NNZ).

Strategy:
- Pure data parallel over the batch (environments) axis: 8 NeuronCores, 500 envs each.
- Per core, the shard is staged host-side into an m-major layout
  [128 partitions, M * ROWS] where column block m holds that m-index's value for
  the partition's 500 (env,f) rows. DMA stays fully contiguous per partition
  (near-peak HBM bandwidth) and every on-chip column access is unit-stride.
- The index/coefficient buffers are tiny and known at kernel-build time, so the
  gather/scatter pattern is specialized into the instruction stream:
  deduplicated (m1,m2) column products plus fused scale+accumulate
  (scalar_tensor_tensor) into per-output-column accumulators on VectorE.
  Common-subexpression merging folds entries sharing (column, output) into
  weighted-sum chains; X1-only chains are hoisted to run in the shadow of the
  X2 load.
- Overlap via column-split tiles (no op splitting, since every op reads whole
  columns): inputs load as an interleaved DMA ladder (CHUNKS columns per step,
  x1c0, x2c0, x1c1, ...) over work-optimized column permutations, with work
  units emitted in input-availability order, so compute starts as soon as the
  first chunks land; the accumulator splits at ACC_BOUNDS with store classes
  scheduled so earlier tiles' stores overlap later classes' compute.
- The host transposes the output back to the reference (row-interleaved) layout.
"""

import numpy as np
from contextlib import ExitStack

B, F, M = 4000, 128, 11
NCORES = 8
BS = B // NCORES            # 500 envs per core
PART = 128
FREE = BS * F * M // PART   # 5500 fp32 per partition
ROWS = FREE // M            # 500 rows per partition
ACC_BOUNDS = [0, 8, 10, 11]  # accumulator tile column boundaries


def _build_plan(m1, m2, mu, C):
    """Group NNZ entries into deduped (a,b)->[(j,c)...] pairs, plus merged
    weighted-sum groups.

    Returns (pairs, merges):
    - pairs: {(a, b): [(j, c), ...]} for entries evaluated as products of
      single columns.
    - merges: [(side, other_m, j, [(m_i, c_i), ...])]: entries sharing one
      (X2-column b, output j) — side 0 — or (X1-column a, j) — side 1 — whose
      column pair is used nowhere else. Evaluated as
      out_j += (sum_i c_i * Xs_{m_i}) * Xo_{other_m}, which costs k+1 ops
      instead of 2k.
    """
    triples = {}
    for a, b, j, c in zip(m1.tolist(), m2.tolist(), mu.tolist(), C.tolist()):
        key = (int(a), int(b), int(j))
        triples[key] = triples.get(key, 0.0) + float(c)
    triples = {k: c for k, c in triples.items() if c != 0.0}

    pair_count = {}
    for a, b, j in triples:
        pair_count[(a, b)] = pair_count.get((a, b), 0) + 1

    by_bj = {}
    by_aj = {}
    for (a, b, j), c in triples.items():
        if pair_count[(a, b)] == 1:
            by_bj.setdefault((b, j), []).append((a, c))
            by_aj.setdefault((a, j), []).append((b, c))

    merges = []
    consumed = set()
    # Greedily take larger groups first; side 0 preferred on ties so the
    # X1-only chains can hide under the X2 DMA.
    cands = [(len(v), 0, bj, v) for bj, v in by_bj.items() if len(v) >= 2]
    cands += [(len(v), 1, aj, v) for aj, v in by_aj.items() if len(v) >= 2]
    cands.sort(key=lambda t: (-t[0], t[1]))
    for _, side, (om, j), entries in cands:
        avail = [
            (m, c)
            for m, c in entries
            if ((m, om) if side == 0 else (om, m)) not in consumed
        ]
        if len(avail) < 2:
            continue
        for m, _ in avail:
            consumed.add((m, om) if side == 0 else (om, m))
        merges.append((side, om, j, avail))

    pairs = {}
    for (a, b, j), c in triples.items():
        if (a, b) in consumed:
            continue
        pairs.setdefault((a, b), []).append((j, c))
    perm1, perm2 = _build_perms(pairs, merges)
    return pairs, merges, perm1, perm2


def _build_perms(pairs, merges):
    """Choose column load orders: the first small chunks of X1/X2 carry the
    columns that unlock the most work, so compute starts right after the
    first chunks land. Greedy per chunk level over CHUNKS sizes."""
    from itertools import combinations

    w = {}
    for (a, b), jl in pairs.items():
        w[(a, b)] = 1 + len(jl)

    perm1, perm2 = [], []
    for n in CHUNKS[:-1]:
        rest_a = [c for c in range(M) if c not in perm1]
        rest_b = [c for c in range(M) if c not in perm2]
        best = (-1, tuple(rest_a[:n]), tuple(rest_b[:n]))
        for A in combinations(rest_a, n):
            Aset = set(perm1) | set(A)
            for Bc in combinations(rest_b, n):
                Bset = set(perm2) | set(Bc)
                s = sum(ww for (a, b), ww in w.items() if a in Aset and b in Bset)
                if s > best[0]:
                    best = (s, A, Bc)
        perm1 += list(best[1])
        perm2 += list(best[2])
    perm1 += [c for c in range(M) if c not in perm1]
    perm2 += [c for c in range(M) if c not in perm2]

    # Within each chunk (membership fixed, internal order free), order the X1
    # columns so pairs sharing an X2 column sit at consecutive staged offsets
    # -> their products fuse into one wide op.
    bsets = {}
    for a, b in pairs:
        bsets.setdefault(a, set()).add(b)

    def order_chunk(cols):
        if len(cols) <= 2:
            return list(cols)
        best_path, best_score = list(cols), -1
        for start in cols:
            path, rem, score = [start], set(cols) - {start}, 0
            while rem:
                nxt = max(
                    rem,
                    key=lambda c: len(
                        bsets.get(path[-1], set()) & bsets.get(c, set())
                    ),
                )
                score += len(bsets.get(path[-1], set()) & bsets.get(nxt, set()))
                path.append(nxt)
                rem.discard(nxt)
            if score > best_score:
                best_path, best_score = path, score
        return best_path

    out1, pos = [], 0
    for n in CHUNKS:
        out1 += order_chunk(perm1[pos : pos + n])
        pos += n
    return out1, perm2


CHUNKS = [1, 1, 2, 3, M - 7]


def _emit_compute(nc, mybir, x1parts, x2parts, acc_parts, scratch_pool,
                  pairs, merges, vec, cmap1, cmap2):
    """Emit the DVE op schedule on permuted, column-chunked tiles.

    x1parts/x2parts: list of tile APs (one per chunk). cmap1/cmap2 map an
    original column index to (part_idx, offset_in_part, avail_rank), where
    avail_rank reflects the interleaved DMA ladder
    x1c0, x2c0, x1c1, x2c1, ... Work units are emitted in availability order
    so compute starts as soon as the first chunks land.
    acc_parts: accumulator tiles split at ACC_BOUNDS.
    """
    mult = mybir.AluOpType.mult
    add = mybir.AluOpType.add

    def col1(m):
        pi, off, _ = cmap1[m]
        return x1parts[pi][:, off * ROWS : (off + 1) * ROWS]

    def col2(m):
        pi, off, _ = cmap2[m]
        return x2parts[pi][:, off * ROWS : (off + 1) * ROWS]

    # acc_parts[t] covers columns [ACC_BOUNDS[t], ACC_BOUNDS[t+1])
    def acc_tile_of(j):
        for t in range(len(ACC_BOUNDS) - 1):
            if j < ACC_BOUNDS[t + 1]:
                return t
        raise ValueError(j)

    def acc_col(j):
        t = acc_tile_of(j)
        off = j - ACC_BOUNDS[t]
        return acc_parts[t][:, off * ROWS : (off + 1) * ROWS]

    init = [False] * M

    # Store-overlap class: the minimum acc tile a unit writes. All writers of
    # tile t run before writers touching only tiles > t, so tile t's store
    # overlaps the remaining compute.
    def class_of_merge(m):
        return acc_tile_of(m[2])

    def class_of_pair(jlist):
        return min(acc_tile_of(j) for j, _ in jlist)

    chain_results = {}

    def emit_chain(mi):
        side, om, j, entries = merges[mi]
        scol = col1 if side == 0 else col2
        s_ap = None
        for i in range(len(entries) - 1):
            m_i, c_i = entries[i]
            m_n, c_n = entries[i + 1]
            src = scol(m_i) if s_ap is None else s_ap
            s_tile = scratch_pool.tile(
                [PART, ROWS], mybir.dt.float32, tag=f"msum{mi}"
            )
            vec.scalar_tensor_tensor(
                s_tile[:], src, c_i / c_n, scol(m_n), op0=mult, op1=add
            )
            s_ap = s_tile[:]
        chain_results[mi] = s_ap

    def emit_merge(mi):
        side, om, j, entries = merges[mi]
        other = col2(om) if side == 0 else col1(om)
        s_ap = chain_results[mi]
        c_last = entries[-1][1]
        if not init[j]:
            vec.scalar_tensor_tensor(
                acc_col(j), s_ap, c_last, other, op0=mult, op1=mult
            )
            init[j] = True
        else:
            p_tile = scratch_pool.tile([PART, ROWS], mybir.dt.float32, tag="msum")
            vec.tensor_tensor(p_tile[:], s_ap, other, op=mult)
            vec.scalar_tensor_tensor(
                acc_col(j), p_tile[:], c_last, acc_col(j), op0=mult, op1=add
            )

    def emit_bootstrap(a, b, jlist):
        """Fused overwrite bootstrap; returns True if handled."""
        j0, c0 = jlist[0]
        if init[j0] or (len(jlist) > 1 and c0 == 0.0):
            return False
        vec.scalar_tensor_tensor(
            acc_col(j0), col1(a), c0, col2(b), op0=mult, op1=mult
        )
        init[j0] = True
        for j, c in jlist[1:]:
            r = c / c0
            if init[j]:
                vec.scalar_tensor_tensor(
                    acc_col(j), acc_col(j0), r, acc_col(j), op0=mult, op1=add
                )
            else:
                nc.scalar.mul(acc_col(j), acc_col(j0), r)
                init[j] = True
        return True

    def emit_pairs(pair_list):
        # Bootstrap pass (fused overwrites) then merged product runs. Runs
        # cover pairs whose m1 columns sit at consecutive staged offsets of
        # the same x1 chunk.
        done = set()
        for a, b in pair_list:
            if emit_bootstrap(a, b, pairs[(a, b)]):
                done.add((a, b))
        rest = [p for p in pair_list if p not in done]
        by_b = {}
        for a, b in rest:
            by_b.setdefault(b, []).append(a)
        for b, alist in by_b.items():
            alist.sort(key=lambda a: (cmap1[a][0], cmap1[a][1]))
            runs = []
            a0 = prev = alist[0]
            for a in alist[1:]:
                if (
                    cmap1[a][0] == cmap1[prev][0]
                    and cmap1[a][1] == cmap1[prev][1] + 1
                ):
                    prev = a
                    continue
                runs.append((a0, prev))
                a0 = prev = a
            runs.append((a0, prev))
            for a0, a1 in runs:
                pi, off0, _ = cmap1[a0]
                g = cmap1[a1][1] - off0 + 1
                members = sorted(
                    (a for a in alist if cmap1[a][0] == pi
                     and off0 <= cmap1[a][1] <= cmap1[a1][1]),
                    key=lambda a: cmap1[a][1],
                )
                prod = scratch_pool.tile(
                    [PART, g * ROWS], mybir.dt.float32, tag="prod"
                )
                in0 = x1parts[pi][:, off0 * ROWS : (off0 + g) * ROWS]
                in1 = (
                    col2(b)
                    .rearrange("p (g r) -> p g r", g=1)
                    .broadcast_to([PART, g, ROWS])
                )
                vec.tensor_tensor(
                    prod[:].rearrange("p (g r) -> p g r", g=g), in0, in1, op=mult
                )
                for a in members:
                    gi = cmap1[a][1] - off0
                    pcol = prod[:, gi * ROWS : (gi + 1) * ROWS]
                    for j, c in pairs[(a, b)]:
                        if init[j]:
                            vec.scalar_tensor_tensor(
                                acc_col(j), pcol, c, acc_col(j), op0=mult, op1=add
                            )
                        else:
                            nc.scalar.mul(acc_col(j), pcol, c)
                            init[j] = True

    # Work units ordered by (late-class, availability rank, kind): chains
    # (which read a single tensor) unlock earliest; later store classes
    # only) block runs last so acc_lo's store overlaps it.
    units = []
    for mi, m in enumerate(merges):
        side, om, j, entries = m
        cmap_s = cmap1 if side == 0 else cmap2
        cmap_o = cmap2 if side == 0 else cmap1
        crank = max(cmap_s[mm][2] for mm, _ in entries)
        frank = max(crank, cmap_o[om][2])
        units.append((0, crank, 0, mi))
        units.append((class_of_merge(m), frank, 1, mi))
    for p, jl in pairs.items():
        a, b = p
        st = max(cmap1[a][2], cmap2[b][2])
        units.append((class_of_pair(jl), st, 2, p))
    units.sort(key=lambda u: (u[0], u[1], u[2]))

    pair_batch = []

    def flush_pairs():
        if pair_batch:
            emit_pairs(list(pair_batch))
            pair_batch.clear()

    for late, stage, kind, payload in units:
        if kind == 2:
            pair_batch.append(payload)
            continue
        flush_pairs()
        if kind == 0:
            emit_chain(payload)
        else:
            emit_merge(payload)
    flush_pairs()

    # Zero any output column no entry maps to.
    for j in range(M):
        if not init[j]:
            vec.memset(acc_col(j), 0.0)


def _build_program(plan, repeat=1):
    import concourse.bass as bass
    import concourse.tile as tile
    from concourse import bacc, mybir

    pairs, merges, perm1, perm2 = plan
    nc = bacc.Bacc(
        "TRN2",
        target_bir_lowering=False,
        debug=False,
        enable_asserts=True,
        num_devices=NCORES,
    )
    x1_d = nc.dram_tensor("x1", [PART, FREE], mybir.dt.float32, kind="ExternalInput").ap()
    x2_d = nc.dram_tensor("x2", [PART, FREE], mybir.dt.float32, kind="ExternalInput").ap()
    out_d = nc.dram_tensor("out", [PART, FREE], mybir.dt.float32, kind="ExternalOutput").ap()

    # cmap: original column -> (part_idx, offset_in_part, avail_rank) with the
    # interleaved DMA ladder x1c0, x2c0, x1c1, x2c1, ...
    def build_cmap(perm, tensor_idx):
        cmap = {}
        pos = 0
        for pi, n in enumerate(CHUNKS):
            for off in range(n):
                cmap[perm[pos]] = (pi, off, 2 * pi + tensor_idx)
                pos += 1
        return cmap

    cmap1 = build_cmap(perm1, 0)
    cmap2 = build_cmap(perm2, 1)

    with ExitStack() as ctx:
        tc = ctx.enter_context(tile.TileContext(nc))
        io_pool = ctx.enter_context(tc.tile_pool(name="io", bufs=1))
        scratch_pool = ctx.enter_context(tc.tile_pool(name="scratch", bufs=2))

        x1parts, x2parts = [], []
        pos = 0
        for pi, n in enumerate(CHUNKS):
            sl = slice(pos * ROWS, (pos + n) * ROWS)
            t1 = io_pool.tile([PART, n * ROWS], mybir.dt.float32, tag=f"x1c{pi}")
            t2 = io_pool.tile([PART, n * ROWS], mybir.dt.float32, tag=f"x2c{pi}")
            x1parts.append((t1, sl))
            x2parts.append((t2, sl))
            pos += n
        for (t1, sl1), (t2, sl2) in zip(x1parts, x2parts):
            nc.sync.dma_start(t1[:], x1_d[:, sl1])
            nc.sync.dma_start(t2[:], x2_d[:, sl2])
        x1aps = [t[:] for t, _ in x1parts]
        x2aps = [t[:] for t, _ in x2parts]

        acc_tiles = []
        for t in range(len(ACC_BOUNDS) - 1):
            n = ACC_BOUNDS[t + 1] - ACC_BOUNDS[t]
            at = io_pool.tile([PART, n * ROWS], mybir.dt.float32, tag=f"acc{t}")
            acc_tiles.append(at)

        for _ in range(repeat):
            _emit_compute(
                nc, mybir, x1aps, x2aps, [a[:] for a in acc_tiles],
                scratch_pool, pairs, merges, nc.vector, cmap1, cmap2,
            )

        # Each acc tile's store depends only on its own writers, so earlier
        # tiles' stores overlap the later classes' compute.
        for t, at in enumerate(acc_tiles):
            lo, hi = ACC_BOUNDS[t] * ROWS, ACC_BOUNDS[t + 1] * ROWS
            nc.sync.dma_start(out_d[:, lo:hi], at[:])

    nc.compile()
    return nc


TRACE = False
LAST_EXEC_NS = None
LAST_TRACE_DIR = None


def _to_mmajor(shard, perm):
    # [BS, F, M] -> [PART, M*ROWS]: staged column block k holds original
    # column perm[k] for the partition's 500 rows.
    return np.ascontiguousarray(
        shard.reshape(PART, ROWS, M).transpose(0, 2, 1)[:, perm, :].reshape(PART, FREE)
    )


def _from_mmajor(flat):
    # inverse of _to_mmajor back to [BS, F, M]
    return np.ascontiguousarray(
        flat.reshape(PART, M, ROWS).transpose(0, 2, 1).reshape(BS, F, M)
    )


def kernel(X1, X2, m1, m2, mu, C):
    global LAST_EXEC_NS, LAST_TRACE_DIR
    from concourse.bass_utils import run_bass_kernel_spmd

    X1 = np.ascontiguousarray(np.asarray(X1, dtype=np.float32))
    X2 = np.ascontiguousarray(np.asarray(X2, dtype=np.float32))
    plan = _build_plan(np.asarray(m1), np.asarray(m2), np.asarray(mu), np.asarray(C))

    nc = _build_program(plan)

    in_maps = []
    for i in range(NCORES):
        sl = slice(i * BS, (i + 1) * BS)
        in_maps.append(
            {
                "x1": _to_mmajor(X1[sl], plan[2]),
                "x2": _to_mmajor(X2[sl], plan[3]),
            }
        )

    kwargs = {}
    if TRACE:
        import tempfile

        LAST_TRACE_DIR = tempfile.mkdtemp(prefix="bass_trace_")
        kwargs = dict(trace=True, tmpdir=LAST_TRACE_DIR)
    res = run_bass_kernel_spmd(nc, in_maps, list(range(NCORES)), **kwargs)
    LAST_EXEC_NS = res.exec_time_ns
    shards = [_from_mmajor(res.results[i]["out"]) for i in range(NCORES)]
    return np.concatenate(shards, axis=0)



# revision 3
# speedup vs baseline: 1.6131x; 1.6131x over previous
"""Trainium2 Bass kernel for CGCalculatorSingle (segment_reduce) — v3.

out[b,f,mu[k]] += C[k] * X1[b,f,m1[k]] * X2[b,f,m2[k]]

Design:
- Pure data parallel: 8 cores x 500 envs; rows (env,f) on 128 partitions,
  m-major columns of 500 rows staged host-side; fp16 on device.
- Three compute engines balanced by a greedy projected-load planner:
  DVE (TT 2x / TS 4x), Act (scaled copies), Pool (self-contained STT).
- Per-(j, engine) partial accumulators: each engine accumulates into its own
  column (first write is a cheap overwrite), so no cross-engine RAW chains;
  partials are combined into acc_j right after the last writer.
- Products are raw pair products on DVE (wide run-merged TT at 2x with
  mid-dim broadcast), shared across all j's of a pair.
- DMA: first column chunks issued from the (idle) DVE/Act queues at t=0,
  the rest from SP; output stored in fine-grained column groups so the tail
  store is short.
"""

import numpy as np
from contextlib import ExitStack

B, F, M = 4000, 128, 11
NCORES = 8
BS = B // NCORES
PART = 128
FREE = BS * F * M // PART
ROWS = FREE // M
ACC_BOUNDS = [0, 3, 6, 8, 10, 11]
NOPART = False
SUBBATCH = 6
CHUNKS = [1, 1, 2, 2, 5]

COST = {
    ("DVE", "TT"): 320.0,
    ("DVE", "TS"): 190.0,
    ("DVE", "STT"): 580.0,
    ("Act", "TS"): 602.0,
    ("Pool", "TT"): 1030.0,
    ("Pool", "TS"): 790.0,
}


def _dedup_triples(m1, m2, mu, C):
    triples = {}
    for a, b, j, c in zip(m1.tolist(), m2.tolist(), mu.tolist(), C.tolist()):
        key = (int(a), int(b), int(j))
        triples[key] = triples.get(key, 0.0) + float(c)
    return {k: c for k, c in triples.items() if c != 0.0}


def _build_plan(m1, m2, mu, C):
    triples = _dedup_triples(np.asarray(m1), np.asarray(m2), np.asarray(mu), np.asarray(C))

    pair_count = {}
    for a, b, j in triples:
        pair_count[(a, b)] = pair_count.get((a, b), 0) + 1

    by_bj = {}
    by_aj = {}
    for (a, b, j), c in triples.items():
        if pair_count[(a, b)] == 1:
            by_bj.setdefault((b, j), []).append((a, c))
            by_aj.setdefault((a, j), []).append((b, c))

    merges = []
    consumed = set()
    cands = [(len(v), 0, bj, v) for bj, v in by_bj.items() if len(v) >= 3]
    cands += [(len(v), 1, aj, v) for aj, v in by_aj.items() if len(v) >= 3]
    cands.sort(key=lambda t: (-t[0], t[1]))
    for _, side, (om, j), entries in cands:
        avail = [
            (m, c)
            for m, c in entries
            if ((m, om) if side == 0 else (om, m)) not in consumed
        ]
        if len(avail) < 3:
            continue
        for m, _ in avail:
            consumed.add((m, om) if side == 0 else (om, m))
        merges.append((side, om, j, avail))

    pairs = {}
    for (a, b, j), c in triples.items():
        if (a, b) in consumed:
            continue
        pairs.setdefault((a, b), []).append((j, c))
    perm1, perm2 = _build_perms(pairs, merges)
    return pairs, merges, perm1, perm2


def _build_perms(pairs, merges):
    from itertools import combinations

    w = {}
    for (a, b), jl in pairs.items():
        w[(a, b)] = 1 + len(jl)

    perm1, perm2 = [], []
    for n in CHUNKS[:-1]:
        rest_a = [c for c in range(M) if c not in perm1]
        rest_b = [c for c in range(M) if c not in perm2]
        best = (-1, tuple(rest_a[:n]), tuple(rest_b[:n]))
        for A in combinations(rest_a, n):
            Aset = set(perm1) | set(A)
            for Bc in combinations(rest_b, n):
                Bset = set(perm2) | set(Bc)
                s = sum(ww for (a, b), ww in w.items() if a in Aset and b in Bset)
                if s > best[0]:
                    best = (s, A, Bc)
        perm1 += list(best[1])
        perm2 += list(best[2])
    perm1 += [c for c in range(M) if c not in perm1]
    perm2 += [c for c in range(M) if c not in perm2]

    bsets = {}
    for a, b in pairs:
        bsets.setdefault(a, set()).add(b)

    def order_chunk(cols):
        if len(cols) <= 2:
            return list(cols)
        best_path, best_score = list(cols), -1
        for start in cols:
            path, rem, score = [start], set(cols) - {start}, 0
            while rem:
                nxt = max(
                    rem,
                    key=lambda c: len(bsets.get(path[-1], set()) & bsets.get(c, set())),
                )
                score += len(bsets.get(path[-1], set()) & bsets.get(nxt, set()))
                path.append(nxt)
                rem.discard(nxt)
            if score > best_score:
                best_path, best_score = path, score
        return best_path

    out1, pos = [], 0
    for n in CHUNKS:
        out1 += order_chunk(perm1[pos : pos + n])
        pos += n
    return out1, perm2


class _Balancer:
    def __init__(self):
        self.load = {"DVE": 0.0, "Act": 0.0, "Pool": 0.0}

    def pick(self, cands):
        best_key, best_val = None, None
        for key, opslist in cands:
            trial = dict(self.load)
            for eng, kind in opslist:
                trial[eng] += COST[(eng, kind)]
            val = (max(trial.values()), sum(trial.values()))
            if best_val is None or val < best_val:
                best_val, best_key = val, key
        return best_key

    def commit(self, opslist):
        for eng, kind in opslist:
            self.load[eng] += COST[(eng, kind)]


def _emit_compute(nc, mybir, x1parts, x2parts, acc_parts, scratch_pool,
                  pairs, merges, cmap1, cmap2):
    mult = mybir.AluOpType.mult
    add = mybir.AluOpType.add
    vec = nc.vector
    act = nc.scalar
    gps = nc.gpsimd
    bal = _Balancer()

    def col1(m):
        pi, off, _ = cmap1[m]
        return x1parts[pi][:, off * ROWS : (off + 1) * ROWS]

    def col2(m):
        pi, off, _ = cmap2[m]
        return x2parts[pi][:, off * ROWS : (off + 1) * ROWS]

    def acc_tile_of(j):
        for t in range(len(ACC_BOUNDS) - 1):
            if j < ACC_BOUNDS[t + 1]:
                return t
        raise ValueError(j)

    def acc_col(j):
        t = acc_tile_of(j)
        off = j - ACC_BOUNDS[t]
        return acc_parts[t][:, off * ROWS : (off + 1) * ROWS]

    # partial state per j: acc ("main", on any engine via overwrite-first) plus
    # per-engine extra partial tiles.  part[j][eng] = (ap, initialized)
    main_init = [False] * M            # acc_col(j) written?
    extra = {}                         # (j, eng) -> AP of partial tile
    _tagn = {"sc": 0, "ch": 0, "prod": 0}

    def rtag(kind, mod):
        _tagn[kind] += 1
        return f"{kind}{_tagn[kind] % mod}"

    def emit_ts(eng, dst, src, c):
        if eng == "DVE":
            vec.tensor_scalar(dst, src, float(c), None, op0=mult)
        elif eng == "Act":
            act.mul(dst, src, float(c))
        else:
            gps.tensor_scalar(dst, src, float(c), None, op0=mult)
        bal.commit([(eng, "TS")])

    def emit_stt(eng, dst, in0, c, in1, op1):
        assert eng == "DVE"
        vec.scalar_tensor_tensor(dst, in0, float(c), in1, op0=mult, op1=op1)
        bal.commit([("DVE", "STT")])

    def emit_tt_add(eng, dst, in0, in1):
        if eng == "Pool":
            gps.tensor_tensor(dst, in0, in1, op=add)
            bal.commit([("Pool", "TT")])
        else:
            vec.tensor_tensor(dst, in0, in1, op=add)
            bal.commit([("DVE", "TT")])

    def target_for(j, eng):
        """Accumulation target for engine eng writing to j.
        Returns (ap, initialized). Claims acc_col(j) if free."""
        if not main_init[j]:
            return acc_col(j), False, "main"
        if not NOPART:
            key = (j, eng)
            if key in extra:
                return extra[key], True, "extra"
            t = scratch_pool.tile([PART, ROWS], mybir.dt.float16, tag=f"px{j}_{eng}")
            extra[key] = t[:]
            return t[:], False, "extra"
        return acc_col(j), True, "main"

    def mark_init(j, kind):
        if kind == "main":
            main_init[j] = True

    # ---------- scatter: target += c * prod ----------
    def scatter(prod_ap, c, j):
        key = bal.pick([
            ("dve", [("DVE", "STT")]),
            ("dve2", [("DVE", "TS"), ("DVE", "TT")]),
            ("act", [("Act", "TS"), ("DVE", "TT")]),
            ("actp", [("Act", "TS"), ("Pool", "TT")]),
            ("poolp", [("Pool", "TS"), ("Pool", "TT")]),
            ("poold", [("Pool", "TS"), ("DVE", "TT")]),
        ])
        if key == "dve":
            tgt, inited, kind = target_for(j, "DVE")
            if inited:
                emit_stt("DVE", tgt, prod_ap, c, tgt, add)
            else:
                emit_ts("DVE", tgt, prod_ap, c)
                mark_init(j, kind)
        elif key == "dve2":
            tgt, inited, kind = target_for(j, "DVE")
            if inited:
                t = scratch_pool.tile([PART, ROWS], mybir.dt.float16,
                                      tag=rtag("sc", 20))
                emit_ts("DVE", t[:], prod_ap, c)
                emit_tt_add("DVE", tgt, t[:], tgt)
            else:
                emit_ts("DVE", tgt, prod_ap, c)
                mark_init(j, kind)
        else:
            scaler = "Act" if key in ("act", "actp") else "Pool"
            adder = "DVE" if key in ("act", "poold") else "Pool"
            tgt, inited, kind = target_for(j, adder)
            if inited:
                t = scratch_pool.tile([PART, ROWS], mybir.dt.float16,
                                      tag=rtag("sc", 20))
                emit_ts(scaler, t[:], prod_ap, c)
                emit_tt_add(adder, tgt, t[:], tgt)
            else:
                emit_ts(scaler, tgt, prod_ap, c)
                mark_init(j, kind)

    # ---------- boot: direct (X1a*c).X2b with no product tile ----------
    def boot_stt(a, b, c, j):
        tgt, inited, kind = target_for(j, "DVE")
        assert not inited
        emit_stt("DVE", tgt, col1(a), c, col2(b), mult)
        mark_init(j, kind)

    # ---------- chains ----------
    chain_results = {}
    chain_eng = {}

    def emit_chain(mi):
        side, om, j, entries = merges[mi]
        scol = col1 if side == 0 else col2
        eng = "DVE"
        chain_eng[mi] = eng
        s_ap = None
        for i in range(len(entries) - 1):
            m_i, c_i = entries[i]
            m_n, c_n = entries[i + 1]
            src = scol(m_i) if s_ap is None else s_ap
            s_tile = scratch_pool.tile([PART, ROWS], mybir.dt.float16,
                                       tag=rtag("ch", 6))
            emit_stt(eng, s_tile[:], src, c_i / c_n, scol(m_n), add)
            s_ap = s_tile[:]
        chain_results[mi] = s_ap

    def emit_merge(mi):
        side, om, j, entries = merges[mi]
        other = col2(om) if side == 0 else col1(om)
        s_ap = chain_results[mi]
        c_last = entries[-1][1]
        tgt, inited, kind = target_for(j, "DVE")
        if not inited:
            emit_stt("DVE", tgt, s_ap, c_last, other, mult)
            mark_init(j, kind)
        else:
            t = scratch_pool.tile([PART, ROWS], mybir.dt.float16, tag=rtag("sc", 20))
            emit_stt("DVE", t[:], s_ap, c_last, other, mult)
            ce = bal.pick([("DVE", [("DVE", "TT")]), ("Pool", [("Pool", "TT")])])
            emit_tt_add(ce, tgt, t[:], tgt)

    # ---------- products ----------
    prod_cols = {}

    def emit_products(pair_list):
        by_b = {}
        for a, b in pair_list:
            by_b.setdefault(b, []).append(a)
        for b, alist in by_b.items():
            alist.sort(key=lambda a: (cmap1[a][0], cmap1[a][1]))
            runs = []
            a0 = prev = alist[0]
            for a in alist[1:]:
                if cmap1[a][0] == cmap1[prev][0] and cmap1[a][1] == cmap1[prev][1] + 1:
                    prev = a
                    continue
                runs.append((a0, prev))
                a0 = prev = a
            runs.append((a0, prev))
            for a0, a1 in runs:
                pi, off0, _ = cmap1[a0]
                g = cmap1[a1][1] - off0 + 1
                members = sorted(
                    (a for a in alist if cmap1[a][0] == pi
                     and off0 <= cmap1[a][1] <= cmap1[a1][1]),
                    key=lambda a: cmap1[a][1],
                )
                prod = scratch_pool.tile([PART, g * ROWS], mybir.dt.float16,
                                         tag=rtag("prod", 20))
                in0 = x1parts[pi][:, off0 * ROWS : (off0 + g) * ROWS]
                in1 = (
                    col2(b)
                    .rearrange("p (g r) -> p g r", g=1)
                    .broadcast_to([PART, g, ROWS])
                )
                vec.tensor_tensor(
                    prod[:].rearrange("p (g r) -> p g r", g=g), in0, in1, op=mult
                )
                bal.commit([("DVE", "TT")] * g)
                for a in members:
                    gi = cmap1[a][1] - off0
                    prod_cols[(a, b)] = prod[:, gi * ROWS : (gi + 1) * ROWS]

    # ---------- scheduling ----------
    units = []
    for mi, m in enumerate(merges):
        side, om, j, entries = m
        cmap_s = cmap1 if side == 0 else cmap2
        cmap_o = cmap2 if side == 0 else cmap1
        crank = max(cmap_s[mm][2] for mm, _ in entries)
        frank = max(crank, cmap_o[om][2])
        units.append((acc_tile_of(j), crank, 0, mi))
        units.append((acc_tile_of(j), frank, 1, mi))
    for p, jl in pairs.items():
        a, b = p
        st = max(cmap1[a][2], cmap2[b][2])
        late = min(acc_tile_of(j) for j, _ in jl)
        units.append((late, st, 2, p))
    units.sort(key=lambda u: (u[1], u[0], u[2]))

    # per-j pending writer counts (scatters + boots + merge finishes)
    writers_left = [0] * M
    for p, jl in pairs.items():
        for j, _ in jl:
            writers_left[j] += 1
    for side, om, j, entries in merges:
        writers_left[j] += 1

    def done_write(j):
        writers_left[j] -= 1
        if writers_left[j] == 0:
            for eng in ("DVE", "Act", "Pool"):
                ap = extra.pop((j, eng), None)
                if ap is not None:
                    ce = bal.pick([("DVE", [("DVE", "TT")]),
                                   ("Pool", [("Pool", "TT")])])
                    emit_tt_add(ce, acc_col(j), ap, acc_col(j))

    def payload_j(mi):
        return merges[mi][2]

    pair_batch = []

    def flush_pairs():
        if not pair_batch:
            return
        need_prod = []
        for p in pair_batch:
            a, b = p
            jl = pairs[p]
            if len(jl) == 1 and not main_init[jl[0][0]]:
                boot_stt(a, b, jl[0][1], jl[0][0])
                done_write(jl[0][0])
            else:
                need_prod.append(p)
        for i0 in range(0, len(need_prod), 6):
            sub = need_prod[i0 : i0 + 6]
            emit_products(sub)
            for p in sub:
                for j, c in sorted(pairs[p], key=lambda jc: main_init[jc[0]]):
                    scatter(prod_cols[p], c, j)
                    done_write(j)
        pair_batch.clear()

    for late, stage, kind, payload in units:
        if kind == 2:
            pair_batch.append(payload)
            continue
        flush_pairs()
        if kind == 0:
            emit_chain(payload)
        else:
            emit_merge(payload)
            done_write(payload_j(payload))
    flush_pairs()

    # safety: combine anything left (shouldn't happen)
    for (j, eng), ap in list(extra.items()):
        emit_tt_add("DVE", acc_col(j), ap, acc_col(j))
        del extra[(j, eng)]

    for j in range(M):
        if not main_init[j]:
            gps.memset(acc_col(j), 0.0)

    return bal.load


def _build_program(plan, verbose=False):
    import concourse.tile as tile
    from concourse import bacc, mybir

    pairs, merges, perm1, perm2 = plan
    nc = bacc.Bacc(
        "TRN2",
        target_bir_lowering=False,
        debug=False,
        enable_asserts=True,
        num_devices=NCORES,
    )
    xin_d = nc.dram_tensor("xin", [PART, 2 * FREE], mybir.dt.float16, kind="ExternalInput").ap()
    out_d = nc.dram_tensor("out", [PART, FREE], mybir.dt.float16, kind="ExternalOutput").ap()

    def build_cmap(perm, tensor_idx):
        cmap = {}
        pos = 0
        for pi, n in enumerate(CHUNKS):
            for off in range(n):
                cmap[perm[pos]] = (pi, off, 2 * pi + tensor_idx)
                pos += 1
        return cmap

    cmap1 = build_cmap(perm1, 0)
    cmap2 = build_cmap(perm2, 1)

    with ExitStack() as ctx:
        tc = ctx.enter_context(tile.TileContext(nc))
        io_pool = ctx.enter_context(tc.tile_pool(name="io", bufs=1))
        scratch_pool = ctx.enter_context(tc.tile_pool(name="scratch", bufs=2))

        # staged input: per chunk [x1 cols | x2 cols] contiguous, one DMA each
        x1aps, x2aps = [], []
        pos = 0
        for pi, n in enumerate(CHUNKS):
            t = io_pool.tile([PART, 2 * n * ROWS], mybir.dt.float16, tag=f"xc{pi}")
            lo = 2 * pos * ROWS
            nc.sync.dma_start(t[:], xin_d[:, lo : lo + 2 * n * ROWS])
            x1aps.append(t[:, : n * ROWS])
            x2aps.append(t[:, n * ROWS :])
            pos += n

        acc_tiles = []
        for t in range(len(ACC_BOUNDS) - 1):
            n = ACC_BOUNDS[t + 1] - ACC_BOUNDS[t]
            at = io_pool.tile([PART, n * ROWS], mybir.dt.float16, tag=f"acc{t}")
            acc_tiles.append(at)

        loads = _emit_compute(
            nc, mybir, x1aps, x2aps, [a[:] for a in acc_tiles],
            scratch_pool, pairs, merges, cmap1, cmap2,
        )
        if verbose:
            print("projected engine loads (ns):", loads)

        for t, at in enumerate(acc_tiles):
            lo, hi = ACC_BOUNDS[t] * ROWS, ACC_BOUNDS[t + 1] * ROWS
            nc.sync.dma_start(out_d[:, lo:hi], at[:])

    nc.compile()
    return nc


TRACE = False
LAST_EXEC_NS = None
LAST_TRACE_DIR = None


def _to_mmajor(shard, perm):
    return np.ascontiguousarray(
        shard.reshape(PART, ROWS, M).transpose(0, 2, 1)[:, perm, :].reshape(PART, FREE)
    )


def _stage_combined(x1shard, x2shard, perm1, perm2):
    """[PART, 2*FREE]: per chunk [x1 cols | x2 cols]."""
    s1 = _to_mmajor(x1shard, perm1).reshape(PART, M, ROWS)
    s2 = _to_mmajor(x2shard, perm2).reshape(PART, M, ROWS)
    segs = []
    pos = 0
    for n in CHUNKS:
        segs.append(s1[:, pos : pos + n].reshape(PART, n * ROWS))
        segs.append(s2[:, pos : pos + n].reshape(PART, n * ROWS))
        pos += n
    return np.ascontiguousarray(np.concatenate(segs, axis=1))


def _from_mmajor(flat):
    return np.ascontiguousarray(
        flat.reshape(PART, M, ROWS).transpose(0, 2, 1).reshape(BS, F, M)
    )


def kernel(X1, X2, m1, m2, mu, C):
    global LAST_EXEC_NS, LAST_TRACE_DIR
    from concourse.bass_utils import run_bass_kernel_spmd

    X1 = np.ascontiguousarray(np.asarray(X1, dtype=np.float16))
    X2 = np.ascontiguousarray(np.asarray(X2, dtype=np.float16))
    plan = _build_plan(np.asarray(m1), np.asarray(m2), np.asarray(mu), np.asarray(C))

    nc = _build_program(plan)

    in_maps = []
    for i in range(NCORES):
        sl = slice(i * BS, (i + 1) * BS)
        in_maps.append(
            {"xin": _stage_combined(X1[sl], X2[sl], plan[2], plan[3])}
        )

    kwargs = {}
    if TRACE:
        import tempfile

        LAST_TRACE_DIR = tempfile.mkdtemp(prefix="bass_trace_")
        kwargs = dict(trace=True, tmpdir=LAST_TRACE_DIR)
    res = run_bass_kernel_spmd(nc, in_maps, list(range(NCORES)), **kwargs)
    LAST_EXEC_NS = res.exec_time_ns
    shards = [_from_mmajor(res.results[i]["out"]).astype(np.float32) for i in range(NCORES)]
    return np.concatenate(shards, axis=0)
